# revision 9
# baseline (speedup 1.0000x reference)
"""KNIFE entropy regularizer loss on 8 Trainium2 NeuronCores.

reference math (per token n, center k):
    dist_sq[n,k] = max(||x_n||^2 + ||c_k||^2 - 2 x_n.c_k, 0)
    kv[n,k]      = exp(-dist_sq / (2 s_k^2))
    density[n]   = sum_k w_k kv[n,k]
    h            = -mean_n log(density + EPS)
    out          = [BETA*h, (h-TGT)^2, BETA*h + (h-TGT)^2, h]

Sharding: data-parallel over the flattened token axis N = B*S = 8192,
1024 tokens per core.  Each core receives its token shard pre-transposed
to [H=1024, T=1024] so the contraction axis (H) lands on SBUF partitions
— every DMA row is a contiguous 4KB run and the PE contracts over H
directly.  The tiny kernel params are replicated (centers pre-packed on
the host into the [128, 8*10] chunk layout the PE weights want).

Device pipeline per core:
  - 8 SWDGE cast-DMAs: xT chunk [128h, 1024t] fp32 -> bf16 SBUF
  - DVE: square (bf16)
  - PE:  psum[10,1024] += (-2c)^T_chunk @ x_chunk  and  ones^T @ x^2_chunk
         (the ones-matmul broadcasts ||x||^2 into all 10 k-rows, fusing
         the x^2 term into the same accumulator)
  - DVE: dist = max(psum + csq_k, 0)   (csq per-partition scalar)
  - ACT: kv = exp(dist * (-1/(2 s_k^2)))  -> bf16
  - PE:  density[1,1024] = w^T @ kv       (bf16 weights)
  - ACT: ln(density + EPS) with fused free-axis accumulation
  - DMA out: one fp32 partial sum per core
The epilogue runs per 512-token half so it overlaps the other half's
matmuls.  Host reduces the 8 partials and forms the 4 output scalars.
"""

from contextlib import ExitStack

import numpy as np

import concourse.bass as bass
import concourse.tile as tile
from concourse import bacc, mybir
from concourse.bass_utils import run_bass_kernel_spmd

B, S, H, K = 4, 2048, 1024, 10
N = B * S                      # 8192 tokens
NCORES = 8
TPC = N // NCORES              # 1024 tokens per core
HCHUNKS = H // 128             # 8 chunks of 128 partitions
HALF = 512                     # tokens per PSUM bank / epilogue slice
BETA = 1.0
TARGET_ENTROPY = 0.0
EPS = 1e-8

F32 = mybir.dt.float32
BF16 = mybir.dt.bfloat16


def _build_program():
    nc = bacc.Bacc("TRN2", target_bir_lowering=False, debug=False,
                   num_devices=NCORES)

    xT = nc.dram_tensor("xT", [H, TPC], F32, kind="ExternalInput").ap()
    cTp = nc.dram_tensor("cTp", [128, HCHUNKS * K], F32,
                         kind="ExternalInput").ap()
    wv = nc.dram_tensor("wv", [K, 1], F32, kind="ExternalInput").ap()
    sv = nc.dram_tensor("sv", [K, 1], F32, kind="ExternalInput").ap()
    out = nc.dram_tensor("out", [1, 1], F32, kind="ExternalOutput").ap()

    with tile.TileContext(nc) as tc, ExitStack() as ctx:
        _emit(tc, ctx, xT, cTp, wv, sv, out)
    nc.compile()
    return nc


def _emit(tc, ctx, xT, cTp, wv, sv, out):
    nc = tc.nc
    singles = ctx.enter_context(tc.tile_pool(name="singles", bufs=1))
    xbpool = ctx.enter_context(tc.tile_pool(name="xb", bufs=HCHUNKS))
    sqpool = ctx.enter_context(tc.tile_pool(name="sq", bufs=HCHUNKS))
    psum = ctx.enter_context(tc.tile_pool(name="ps", bufs=1, space="PSUM"))

    nhalf = TPC // HALF

    # ---- tiny params (HWDGE loads, all contiguous / tiny) ----
    ct_sb = singles.tile([128, HCHUNKS, K], F32)      # [p, j, k] host-packed
    nc.sync.dma_start(ct_sb[:], cTp.rearrange("p (j k) -> p j k", k=K))
    w_sb = singles.tile([K, 1], F32)
    nc.sync.dma_start(w_sb[:], wv[:, :])
    s_sb = singles.tile([K, 1], F32)
    nc.sync.dma_start(s_sb[:], sv[:, :])

    # ---- constants ----
    ones_bf = singles.tile([128, K], BF16)
    nc.vector.memset(ones_bf[:], 1.0)
    ones_f1 = singles.tile([128, 1], F32)
    nc.vector.memset(ones_f1[:], 1.0)
    zero_k = singles.tile([K, 1], F32)
    nc.vector.memset(zero_k[:], 0.0)
    eps_sb = singles.tile([1, 1], F32)
    nc.vector.memset(eps_sb[:], EPS)

    # ---- derived params (all tiny; off the hot path) ----
    c2_bf = singles.tile([128, HCHUNKS, K], BF16)     # -2c as bf16 weights
    nc.vector.tensor_scalar_mul(c2_bf[:], ct_sb[:], -2.0)
    w_bf = singles.tile([K, 1], BF16)
    nc.vector.tensor_copy(w_bf[:], w_sb[:])

    # -1/(2 s^2) per-partition scalar
    s2 = singles.tile([K, 1], F32)
    nc.vector.tensor_mul(s2[:], s_sb[:], s_sb[:])
    nc.vector.tensor_scalar_mul(s2[:], s2[:], 2.0)
    ninv = singles.tile([K, 1], F32)
    nc.vector.reciprocal(ninv[:], s2[:])
    nc.vector.tensor_scalar_mul(ninv[:], ninv[:], -1.0)

    # c_sq[k] = sum_h c[k,h]^2 -> [K,1] per-partition scalar
    sqc = singles.tile([128, HCHUNKS * K], F32)
    ct_flat = ct_sb.rearrange("p j k -> p (j k)")
    nc.vector.tensor_mul(sqc[:], ct_flat, ct_flat)
    ps_csq = psum.tile([1, HCHUNKS * K], F32)
    nc.tensor.matmul(ps_csq[:], lhsT=ones_f1[:], rhs=sqc[:],
                     start=True, stop=True)
    csq_row = singles.tile([1, K], F32)
    nc.vector.tensor_reduce(
        csq_row[:], ps_csq.rearrange("p (j k) -> p k j", j=HCHUNKS),
        axis=mybir.AxisListType.X, op=mybir.AluOpType.add)
    ps_csqT = psum.tile([K, 1], F32)
    nc.tensor.matmul(ps_csqT[:], lhsT=csq_row[:], rhs=ones_f1[0:1, 0:1],
                     start=True, stop=True)
    csqT = singles.tile([K, 1], F32)
    nc.scalar.copy(csqT[:], ps_csqT[:])

    # ---- preload the Ln activation table off the critical tail ----
    lnscratch = singles.tile([1, 1], F32)
    nc.scalar.activation(lnscratch[:], eps_sb[:],
                         mybir.ActivationFunctionType.Ln, bias=eps_sb[:])

    # ---- stream x in fp32 over HWDGE; issue ALL dmas before any compute
    # lands on the issuing engines (no head-of-line blocking) ----
    xtpool = ctx.enter_context(tc.tile_pool(name="xt", bufs=HCHUNKS))
    xt = []
    for j in range(HCHUNKS):
        xtj = xtpool.tile([128, TPC], F32)
        dma_eng = nc.sync if j % 2 == 0 else nc.scalar
        dma_eng.dma_start(xtj[:], xT[j * 128:(j + 1) * 128, :])
        xt.append(xtj)

    # casts split DVE/ACT; squares on DVE read the fp32 tile directly so
    # cast and square run concurrently on different engines
    xb = []
    sq = []
    for j in range(HCHUNKS):
        xbj = xbpool.tile([128, TPC], BF16)
        if j % 2 == 0:
            nc.vector.tensor_copy(xbj[:], xt[j][:])
        else:
            nc.scalar.copy(xbj[:], xt[j][:])
        sqj = sqpool.tile([128, TPC], BF16)
        nc.vector.tensor_mul(sqj[:], xt[j][:], xt[j][:])
        xb.append(xbj)
        sq.append(sqj)

    # ---- main accumulation: psum[k, t] = x_sq[t] - 2 dot[k, t] ----
    ps_dist = psum.tile([K, TPC], F32)
    for h in range(nhalf):
        sl = slice(h * HALF, (h + 1) * HALF)
        for j in range(HCHUNKS):
            nc.tensor.matmul(ps_dist[:, sl], lhsT=c2_bf[:, j, :],
                             rhs=xb[j][:, sl], start=(j == 0), stop=False)
            nc.tensor.matmul(ps_dist[:, sl], lhsT=ones_bf[:],
                             rhs=sq[j][:, sl], start=False,
                             stop=(j == HCHUNKS - 1))

    # ---- per-half epilogue ----
    kv = []
    for h in range(nhalf):
        sl = slice(h * HALF, (h + 1) * HALF)
        dist = singles.tile([K, HALF], F32, tag=f"dist{h}")
        nc.vector.tensor_scalar(dist[:], ps_dist[:, sl], scalar1=csqT[:],
                                scalar2=0.0, op0=mybir.AluOpType.add,
                                op1=mybir.AluOpType.max)
        kvh = singles.tile([K, HALF], BF16, tag=f"kv{h}")
        nc.scalar.activation(kvh[:], dist[:], mybir.ActivationFunctionType.Exp,
                             bias=zero_k[:], scale=ninv[:])
        kv.append(kvh)

    ps_dens = psum.tile([1, TPC], F32)
    for h in range(nhalf):
        sl = slice(h * HALF, (h + 1) * HALF)
        nc.tensor.matmul(ps_dens[:, sl], lhsT=w_bf[:], rhs=kv[h][:],
                         start=True, stop=True)

    ld = singles.tile([1, TPC], F32)
    ldsum = singles.tile([1, 1], F32)
    nc.scalar.activation(ld[:], ps_dens[:], mybir.ActivationFunctionType.Ln,
                         bias=eps_sb[:], accum_out=ldsum[:])
    nc.sync.dma_start(out[:, :], ldsum[:])


def _make_in_maps(hidden_states, kernel_centers, kernel_weights, kernel_scales):
    h_flat = np.asarray(hidden_states, dtype=np.float32).reshape(N, H)
    c = np.asarray(kernel_centers, np.float32)
    # [p, j, k] chunk layout: cTp[p, j*K+k] = c[k, j*128+p]
    cTp = np.ascontiguousarray(
        c.T.reshape(HCHUNKS, 128, K).transpose(1, 0, 2).reshape(128,
                                                                HCHUNKS * K))
    wv = np.asarray(kernel_weights, np.float32).reshape(K, 1)
    sv = np.asarray(kernel_scales, np.float32).reshape(K, 1)
    in_maps = []
    for core in range(NCORES):
        shard = h_flat[core * TPC:(core + 1) * TPC, :]    # [TPC, H]
        in_maps.append({
            "xT": np.ascontiguousarray(shard.T),          # [H, TPC]
            "cTp": cTp,
            "wv": wv,
            "sv": sv,
        })
    return in_maps


def run(inputs, trace=False, **run_kwargs):
    """Compile + run on 8 cores. Returns (output[4], BassKernelResults)."""
    nc = _build_program()
    in_maps = _make_in_maps(**inputs)
    results = run_bass_kernel_spmd(
        nc, in_maps, core_ids=list(range(NCORES)), trace=trace, **run_kwargs)
    partial = np.float32(0.0)
    for r in results.results:
        partial += np.float32(r["out"][0, 0])
    h = np.float32(-(partial / np.float32(N)))
    entropy_loss = np.float32(BETA) * h
    target_entropy_loss = np.float32((h - TARGET_ENTROPY) ** 2)
    total_loss = entropy_loss + target_entropy_loss
    outv = np.stack([entropy_loss, target_entropy_loss, total_loss, h]).astype(
        np.float32)
    return outv, results


def kernel(**inputs):
    outv, _ = run(inputs, trace=False)
    return outv


# revision 10
# speedup vs baseline: 1.0728x; 1.0728x over previous
"""KNIFE entropy regularizer loss on 8 Trainium2 NeuronCores.

reference math (per token n, center k):
    dist_sq[n,k] = max(||x_n||^2 + ||c_k||^2 - 2 x_n.c_k, 0)
    kv[n,k]      = exp(-dist_sq / (2 s_k^2))
    density[n]   = sum_k w_k kv[n,k]
    h            = -mean_n log(density + EPS)
    out          = [BETA*h, (h-TGT)^2, BETA*h + (h-TGT)^2, h]

Sharding: data-parallel over the flattened token axis N = B*S = 8192,
1024 tokens per core.  Each core receives its token shard pre-transposed
to [H=1024, T=1024] so the contraction axis (H) lands on SBUF partitions
— every DMA row is a contiguous 4KB run and the PE contracts over H
directly.  The tiny kernel params are replicated (centers pre-packed on
the host into the [128, 8*10] chunk layout the PE weights want).

Device pipeline per core:
  - 8 SWDGE cast-DMAs: xT chunk [128h, 1024t] fp32 -> bf16 SBUF
  - DVE: square (bf16)
  - PE:  psum[10,1024] += (-2c)^T_chunk @ x_chunk  and  ones^T @ x^2_chunk
         (the ones-matmul broadcasts ||x||^2 into all 10 k-rows, fusing
         the x^2 term into the same accumulator)
  - DVE: dist = max(psum + csq_k, 0)   (csq per-partition scalar)
  - ACT: kv = exp(dist * (-1/(2 s_k^2)))  -> bf16
  - PE:  density[1,1024] = w^T @ kv       (bf16 weights)
  - ACT: ln(density + EPS) with fused free-axis accumulation
  - DMA out: one fp32 partial sum per core
The epilogue runs per 512-token half so it overlaps the other half's
matmuls.  Host reduces the 8 partials and forms the 4 output scalars.
"""

from contextlib import ExitStack

import numpy as np

import concourse.bass as bass
import concourse.tile as tile
from concourse import bacc, mybir
from concourse.bass_utils import run_bass_kernel_spmd

B, S, H, K = 4, 2048, 1024, 10
N = B * S                      # 8192 tokens
NCORES = 8
TPC = N // NCORES              # 1024 tokens per core
HCHUNKS = H // 128             # 8 chunks of 128 partitions
HALF = 512                     # tokens per PSUM bank / epilogue slice
BETA = 1.0
TARGET_ENTROPY = 0.0
EPS = 1e-8

F32 = mybir.dt.float32
BF16 = mybir.dt.bfloat16


def _build_program():
    nc = bacc.Bacc("TRN2", target_bir_lowering=False, debug=False,
                   num_devices=NCORES)

    xT = nc.dram_tensor("xT", [H, TPC], F32, kind="ExternalInput").ap()
    cTp = nc.dram_tensor("cTp", [128, HCHUNKS * K], F32,
                         kind="ExternalInput").ap()
    wv = nc.dram_tensor("wv", [K, 1], F32, kind="ExternalInput").ap()
    sv = nc.dram_tensor("sv", [K, 1], F32, kind="ExternalInput").ap()
    out = nc.dram_tensor("out", [1, 1], F32, kind="ExternalOutput").ap()

    with tile.TileContext(nc) as tc, ExitStack() as ctx:
        _emit(tc, ctx, xT, cTp, wv, sv, out)
    nc.compile()
    return nc


def _emit(tc, ctx, xT, cTp, wv, sv, out):
    nc = tc.nc
    singles = ctx.enter_context(tc.tile_pool(name="singles", bufs=1))
    xbpool = ctx.enter_context(tc.tile_pool(name="xb", bufs=HCHUNKS))
    sqpool = ctx.enter_context(tc.tile_pool(name="sq", bufs=HCHUNKS))
    psum = ctx.enter_context(tc.tile_pool(name="ps", bufs=1, space="PSUM"))

    nhalf = TPC // HALF

    # ---- tiny params (HWDGE loads, all contiguous / tiny) ----
    ct_sb = singles.tile([128, HCHUNKS, K], F32)      # [p, j, k] host-packed
    nc.sync.dma_start(ct_sb[:], cTp.rearrange("p (j k) -> p j k", k=K))
    w_sb = singles.tile([K, 1], F32)
    nc.sync.dma_start(w_sb[:], wv[:, :])
    s_sb = singles.tile([K, 1], F32)
    nc.sync.dma_start(s_sb[:], sv[:, :])

    # ---- constants ----
    ones_bf = singles.tile([128, K], BF16)
    nc.vector.memset(ones_bf[:], 1.0)
    ones_f1 = singles.tile([128, 1], F32)
    nc.vector.memset(ones_f1[:], 1.0)
    zero_k = singles.tile([K, 1], F32)
    nc.vector.memset(zero_k[:], 0.0)
    eps_sb = singles.tile([1, 1], F32)
    nc.vector.memset(eps_sb[:], EPS)

    # ---- derived params (all tiny; off the hot path) ----
    c2_bf = singles.tile([128, HCHUNKS, K], BF16)     # -2c as bf16 weights
    nc.vector.tensor_scalar_mul(c2_bf[:], ct_sb[:], -2.0)
    w_bf = singles.tile([K, 1], BF16)
    nc.vector.tensor_copy(w_bf[:], w_sb[:])

    # -1/(2 s^2) per-partition scalar
    s2 = singles.tile([K, 1], F32)
    nc.vector.tensor_mul(s2[:], s_sb[:], s_sb[:])
    nc.vector.tensor_scalar_mul(s2[:], s2[:], 2.0)
    ninv = singles.tile([K, 1], F32)
    nc.vector.reciprocal(ninv[:], s2[:])
    nc.vector.tensor_scalar_mul(ninv[:], ninv[:], -1.0)

    # c_sq[k] = sum_h c[k,h]^2 -> [K,1] per-partition scalar
    sqc = singles.tile([128, HCHUNKS * K], F32)
    ct_flat = ct_sb.rearrange("p j k -> p (j k)")
    nc.vector.tensor_mul(sqc[:], ct_flat, ct_flat)
    ps_csq = psum.tile([1, HCHUNKS * K], F32)
    nc.tensor.matmul(ps_csq[:], lhsT=ones_f1[:], rhs=sqc[:],
                     start=True, stop=True)
    csq_row = singles.tile([1, K], F32)
    nc.vector.tensor_reduce(
        csq_row[:], ps_csq.rearrange("p (j k) -> p k j", j=HCHUNKS),
        axis=mybir.AxisListType.X, op=mybir.AluOpType.add)
    ps_csqT = psum.tile([K, 1], F32)
    nc.tensor.matmul(ps_csqT[:], lhsT=csq_row[:], rhs=ones_f1[0:1, 0:1],
                     start=True, stop=True)
    csqT = singles.tile([K, 1], F32)
    nc.scalar.copy(csqT[:], ps_csqT[:])

    # ---- preload BOTH activation tables off the critical tail ----
    lnscratch = singles.tile([1, 1], F32)
    nc.scalar.activation(lnscratch[:], eps_sb[:],
                         mybir.ActivationFunctionType.Ln, bias=eps_sb[:])
    nc.scalar.activation(lnscratch[:], eps_sb[:],
                         mybir.ActivationFunctionType.Exp, bias=eps_sb[:])

    # ---- stream x in: first 3 chunks via SWDGE cast-DMA (bf16 lands
    # directly, separate queue group), rest via HWDGE fp32 + engine casts.
    # All dma issues emitted before engine compute (no head-of-line) ----
    N_SW = 3
    xtpool = ctx.enter_context(tc.tile_pool(name="xt", bufs=HCHUNKS))
    xb = [None] * HCHUNKS
    xt = [None] * HCHUNKS
    for j in range(N_SW):
        xbj = xbpool.tile([128, TPC], BF16)
        nc.gpsimd.dma_start(xbj[:], xT[j * 128:(j + 1) * 128, :])
        xb[j] = xbj
    for j in range(N_SW, HCHUNKS):
        xtj = xtpool.tile([128, TPC], F32)
        dma_eng = nc.sync if (j - N_SW) % 2 == 0 else nc.scalar
        dma_eng.dma_start(xtj[:], xT[j * 128:(j + 1) * 128, :])
        xt[j] = xtj
    for j in range(N_SW, HCHUNKS):
        xbj = xbpool.tile([128, TPC], BF16)
        if (j - N_SW) % 2 == 0:
            nc.vector.tensor_copy(xbj[:], xt[j][:])
        else:
            nc.scalar.copy(xbj[:], xt[j][:])
        xb[j] = xbj
    sq = []
    for j in range(HCHUNKS):
        sqj = sqpool.tile([128, TPC], BF16)
        nc.vector.tensor_mul(sqj[:], xb[j][:], xb[j][:])
        sq.append(sqj)

    # ---- main accumulation: psum[k, t] = x_sq[t] - 2 dot[k, t] ----
    ps_dist = psum.tile([K, TPC], F32)
    for h in range(nhalf):
        sl = slice(h * HALF, (h + 1) * HALF)
        for j in range(HCHUNKS):
            nc.tensor.matmul(ps_dist[:, sl], lhsT=c2_bf[:, j, :],
                             rhs=xb[j][:, sl], start=(j == 0), stop=False)
            nc.tensor.matmul(ps_dist[:, sl], lhsT=ones_bf[:],
                             rhs=sq[j][:, sl], start=False,
                             stop=(j == HCHUNKS - 1))

    # ---- per-half epilogue ----
    kv = []
    for h in range(nhalf):
        sl = slice(h * HALF, (h + 1) * HALF)
        dist = singles.tile([K, HALF], F32, tag=f"dist{h}")
        nc.vector.tensor_scalar(dist[:], ps_dist[:, sl], scalar1=csqT[:],
                                scalar2=0.0, op0=mybir.AluOpType.add,
                                op1=mybir.AluOpType.max)
        kvh = singles.tile([K, HALF], BF16, tag=f"kv{h}")
        nc.scalar.activation(kvh[:], dist[:], mybir.ActivationFunctionType.Exp,
                             bias=zero_k[:], scale=ninv[:])
        kv.append(kvh)

    ps_dens = psum.tile([1, TPC], F32)
    for h in range(nhalf):
        sl = slice(h * HALF, (h + 1) * HALF)
        nc.tensor.matmul(ps_dens[:, sl], lhsT=w_bf[:], rhs=kv[h][:],
                         start=True, stop=True)

    ld = singles.tile([1, TPC], F32)
    ldsum = singles.tile([1, 1], F32)
    nc.scalar.activation(ld[:], ps_dens[:], mybir.ActivationFunctionType.Ln,
                         bias=eps_sb[:], accum_out=ldsum[:])
    nc.sync.dma_start(out[:, :], ldsum[:])


def _make_in_maps(hidden_states, kernel_centers, kernel_weights, kernel_scales):
    h_flat = np.asarray(hidden_states, dtype=np.float32).reshape(N, H)
    c = np.asarray(kernel_centers, np.float32)
    # [p, j, k] chunk layout: cTp[p, j*K+k] = c[k, j*128+p]
    cTp = np.ascontiguousarray(
        c.T.reshape(HCHUNKS, 128, K).transpose(1, 0, 2).reshape(128,
                                                                HCHUNKS * K))
    wv = np.asarray(kernel_weights, np.float32).reshape(K, 1)
    sv = np.asarray(kernel_scales, np.float32).reshape(K, 1)
    in_maps = []
    for core in range(NCORES):
        shard = h_flat[core * TPC:(core + 1) * TPC, :]    # [TPC, H]
        in_maps.append({
            "xT": np.ascontiguousarray(shard.T),          # [H, TPC]
            "cTp": cTp,
            "wv": wv,
            "sv": sv,
        })
    return in_maps


def run(inputs, trace=False, **run_kwargs):
    """Compile + run on 8 cores. Returns (output[4], BassKernelResults)."""
    nc = _build_program()
    in_maps = _make_in_maps(**inputs)
    results = run_bass_kernel_spmd(
        nc, in_maps, core_ids=list(range(NCORES)), trace=trace, **run_kwargs)
    partial = np.float32(0.0)
    for r in results.results:
        partial += np.float32(r["out"][0, 0])
    h = np.float32(-(partial / np.float32(N)))
    entropy_loss = np.float32(BETA) * h
    target_entropy_loss = np.float32((h - TARGET_ENTROPY) ** 2)
    total_loss = entropy_loss + target_entropy_loss
    outv = np.stack([entropy_loss, target_entropy_loss, total_loss, h]).astype(
        np.float32)
    return outv, results


def kernel(**inputs):
    outv, _ = run(inputs, trace=False)
    return outv


# revision 11
# speedup vs baseline: 1.0910x; 1.0170x over previous
"""KNIFE entropy regularizer loss on 8 Trainium2 NeuronCores.

reference math (per token n, center k):
    dist_sq[n,k] = max(||x_n||^2 + ||c_k||^2 - 2 x_n.c_k, 0)
    kv[n,k]      = exp(-dist_sq / (2 s_k^2))
    density[n]   = sum_k w_k kv[n,k]
    h            = -mean_n log(density + EPS)
    out          = [BETA*h, (h-TGT)^2, BETA*h + (h-TGT)^2, h]

Sharding: data-parallel over the flattened token axis N = B*S = 8192,
1024 tokens per core.  Each core receives its token shard pre-transposed
to [H=1024, T=1024] so the contraction axis (H) lands on SBUF partitions
— every DMA row is a contiguous 4KB run and the PE contracts over H
directly.  The tiny kernel params are replicated (centers pre-packed on
the host into the [128, 8*10] chunk layout the PE weights want).

Device pipeline per core:
  - 8 SWDGE cast-DMAs: xT chunk [128h, 1024t] fp32 -> bf16 SBUF
  - DVE: square (bf16)
  - PE:  psum[10,1024] += (-2c)^T_chunk @ x_chunk  and  ones^T @ x^2_chunk
         (the ones-matmul broadcasts ||x||^2 into all 10 k-rows, fusing
         the x^2 term into the same accumulator)
  - DVE: dist = max(psum + csq_k, 0)   (csq per-partition scalar)
  - ACT: kv = exp(dist * (-1/(2 s_k^2)))  -> bf16
  - PE:  density[1,1024] = w^T @ kv       (bf16 weights)
  - ACT: ln(density + EPS) with fused free-axis accumulation
  - DMA out: one fp32 partial sum per core
The epilogue runs per 512-token half so it overlaps the other half's
matmuls.  Host reduces the 8 partials and forms the 4 output scalars.
"""

from contextlib import ExitStack

import numpy as np

import concourse.bass as bass
import concourse.tile as tile
from concourse import bacc, mybir
from concourse.bass_utils import run_bass_kernel_spmd

B, S, H, K = 4, 2048, 1024, 10
N = B * S                      # 8192 tokens
NCORES = 8
TPC = N // NCORES              # 1024 tokens per core
HCHUNKS = H // 128             # 8 chunks of 128 partitions
HALF = 512                     # tokens per PSUM bank / epilogue slice
BETA = 1.0
TARGET_ENTROPY = 0.0
EPS = 1e-8

F32 = mybir.dt.float32
BF16 = mybir.dt.bfloat16


def _build_program():
    nc = bacc.Bacc("TRN2", target_bir_lowering=False, debug=False,
                   num_devices=NCORES)

    xT = nc.dram_tensor("xT", [H, TPC], F32, kind="ExternalInput").ap()
    cTp = nc.dram_tensor("cTp", [128, HCHUNKS * K], F32,
                         kind="ExternalInput").ap()
    wv = nc.dram_tensor("wv", [K, 1], F32, kind="ExternalInput").ap()
    sv = nc.dram_tensor("sv", [K, 1], F32, kind="ExternalInput").ap()
    out = nc.dram_tensor("out", [1, 1], F32, kind="ExternalOutput").ap()

    with tile.TileContext(nc) as tc, ExitStack() as ctx:
        _emit(tc, ctx, xT, cTp, wv, sv, out)
    nc.compile()
    return nc


def _emit(tc, ctx, xT, cTp, wv, sv, out):
    nc = tc.nc
    singles = ctx.enter_context(tc.tile_pool(name="singles", bufs=1))
    xbpool = ctx.enter_context(tc.tile_pool(name="xb", bufs=HCHUNKS))
    sqpool = ctx.enter_context(tc.tile_pool(name="sq", bufs=HCHUNKS))
    psum = ctx.enter_context(tc.tile_pool(name="ps", bufs=1, space="PSUM"))

    nhalf = TPC // HALF

    # ---- tiny params (HWDGE loads, all contiguous / tiny) ----
    ct_sb = singles.tile([128, HCHUNKS, K], F32)      # [p, j, k] host-packed
    nc.sync.dma_start(ct_sb[:], cTp.rearrange("p (j k) -> p j k", k=K))
    w_sb = singles.tile([K, 1], F32)
    nc.sync.dma_start(w_sb[:], wv[:, :])
    s_sb = singles.tile([K, 1], F32)
    nc.sync.dma_start(s_sb[:], sv[:, :])

    # ---- constants ----
    ones_bf = singles.tile([128, K], BF16)
    nc.vector.memset(ones_bf[:], 1.0)
    ones_f1 = singles.tile([128, 1], F32)
    nc.vector.memset(ones_f1[:], 1.0)
    zero_k = singles.tile([K, 1], F32)
    nc.vector.memset(zero_k[:], 0.0)
    eps_sb = singles.tile([1, 1], F32)
    nc.vector.memset(eps_sb[:], EPS)

    # ---- derived params (all tiny; off the hot path) ----
    c2_bf = singles.tile([128, HCHUNKS, K], BF16)     # -2c as bf16 weights
    nc.vector.tensor_scalar_mul(c2_bf[:], ct_sb[:], -2.0)
    w_bf = singles.tile([K, 1], BF16)
    nc.vector.tensor_copy(w_bf[:], w_sb[:])

    # -1/(2 s^2) per-partition scalar
    s2 = singles.tile([K, 1], F32)
    nc.vector.tensor_mul(s2[:], s_sb[:], s_sb[:])
    nc.vector.tensor_scalar_mul(s2[:], s2[:], 2.0)
    ninv = singles.tile([K, 1], F32)
    nc.vector.reciprocal(ninv[:], s2[:])
    nc.vector.tensor_scalar_mul(ninv[:], ninv[:], -1.0)

    # c_sq[k] = sum_h c[k,h]^2 -> [K,1] per-partition scalar
    sqc = singles.tile([128, HCHUNKS * K], F32)
    ct_flat = ct_sb.rearrange("p j k -> p (j k)")
    nc.vector.tensor_mul(sqc[:], ct_flat, ct_flat)
    ps_csq = psum.tile([1, HCHUNKS * K], F32)
    nc.tensor.matmul(ps_csq[:], lhsT=ones_f1[:], rhs=sqc[:],
                     start=True, stop=True)
    csq_row = singles.tile([1, K], F32)
    nc.vector.tensor_reduce(
        csq_row[:], ps_csq.rearrange("p (j k) -> p k j", j=HCHUNKS),
        axis=mybir.AxisListType.X, op=mybir.AluOpType.add)
    ps_csqT = psum.tile([K, 1], F32)
    nc.tensor.matmul(ps_csqT[:], lhsT=csq_row[:], rhs=ones_f1[0:1, 0:1],
                     start=True, stop=True)
    csqT = singles.tile([K, 1], F32)
    nc.scalar.copy(csqT[:], ps_csqT[:])

    # ---- stream x in fp32 over HWDGE; all dma issues emitted before any
    # engine compute (no head-of-line blocking on the issuing engines).
    # Early-arriving chunks cast on ACT (slower, idle engine); the last
    # chunks cast on DVE so the post-wire tail chain is short ----
    xtpool = ctx.enter_context(tc.tile_pool(name="xt", bufs=HCHUNKS))
    xt = []
    for j in range(HCHUNKS):
        xtj = xtpool.tile([128, TPC], F32)
        dma_eng = nc.sync if j % 2 == 0 else nc.scalar
        dma_eng.dma_start(xtj[:], xT[j * 128:(j + 1) * 128, :])
        xt.append(xtj)
    xb = []
    sq = []
    for j in range(HCHUNKS):
        xbj = xbpool.tile([128, TPC], BF16)
        if j < 4:
            nc.scalar.copy(xbj[:], xt[j][:])
        else:
            nc.vector.tensor_copy(xbj[:], xt[j][:])
        sqj = sqpool.tile([128, TPC], BF16)
        nc.vector.tensor_mul(sqj[:], xbj[:], xbj[:])
        xb.append(xbj)
        sq.append(sqj)

    # ---- main accumulation: psum[k, t] = x_sq[t] - 2 dot[k, t] ----
    ps_dist = psum.tile([K, TPC], F32)
    for h in range(nhalf):
        sl = slice(h * HALF, (h + 1) * HALF)
        for j in range(HCHUNKS):
            nc.tensor.matmul(ps_dist[:, sl], lhsT=c2_bf[:, j, :],
                             rhs=xb[j][:, sl], start=(j == 0), stop=False)
            nc.tensor.matmul(ps_dist[:, sl], lhsT=ones_bf[:],
                             rhs=sq[j][:, sl], start=False,
                             stop=(j == HCHUNKS - 1))

    # ---- per-half epilogue ----
    kv = []
    for h in range(nhalf):
        sl = slice(h * HALF, (h + 1) * HALF)
        dist = singles.tile([K, HALF], F32, tag=f"dist{h}")
        nc.vector.tensor_scalar(dist[:], ps_dist[:, sl], scalar1=csqT[:],
                                scalar2=0.0, op0=mybir.AluOpType.add,
                                op1=mybir.AluOpType.max)
        kvh = singles.tile([K, HALF], BF16, tag=f"kv{h}")
        nc.scalar.activation(kvh[:], dist[:], mybir.ActivationFunctionType.Exp,
                             bias=zero_k[:], scale=ninv[:])
        kv.append(kvh)

    ps_dens = psum.tile([1, TPC], F32)
    for h in range(nhalf):
        sl = slice(h * HALF, (h + 1) * HALF)
        nc.tensor.matmul(ps_dens[:, sl], lhsT=w_bf[:], rhs=kv[h][:],
                         start=True, stop=True)

    ld = singles.tile([1, TPC], F32)
    ldsum = singles.tile([1, 1], F32)
    nc.scalar.activation(ld[:], ps_dens[:], mybir.ActivationFunctionType.Ln,
                         bias=eps_sb[:], accum_out=ldsum[:])
    nc.sync.dma_start(out[:, :], ldsum[:])


def _make_in_maps(hidden_states, kernel_centers, kernel_weights, kernel_scales):
    h_flat = np.asarray(hidden_states, dtype=np.float32).reshape(N, H)
    c = np.asarray(kernel_centers, np.float32)
    # [p, j, k] chunk layout: cTp[p, j*K+k] = c[k, j*128+p]
    cTp = np.ascontiguousarray(
        c.T.reshape(HCHUNKS, 128, K).transpose(1, 0, 2).reshape(128,
                                                                HCHUNKS * K))
    wv = np.asarray(kernel_weights, np.float32).reshape(K, 1)
    sv = np.asarray(kernel_scales, np.float32).reshape(K, 1)
    in_maps = []
    for core in range(NCORES):
        shard = h_flat[core * TPC:(core + 1) * TPC, :]    # [TPC, H]
        in_maps.append({
            "xT": np.ascontiguousarray(shard.T),          # [H, TPC]
            "cTp": cTp,
            "wv": wv,
            "sv": sv,
        })
    return in_maps


def run(inputs, trace=False, **run_kwargs):
    """Compile + run on 8 cores. Returns (output[4], BassKernelResults)."""
    nc = _build_program()
    in_maps = _make_in_maps(**inputs)
    results = run_bass_kernel_spmd(
        nc, in_maps, core_ids=list(range(NCORES)), trace=trace, **run_kwargs)
    partial = np.float32(0.0)
    for r in results.results:
        partial += np.float32(r["out"][0, 0])
    h = np.float32(-(partial / np.float32(N)))
    entropy_loss = np.float32(BETA) * h
    target_entropy_loss = np.float32((h - TARGET_ENTROPY) ** 2)
    total_loss = entropy_loss + target_entropy_loss
    outv = np.stack([entropy_loss, target_entropy_loss, total_loss, h]).astype(
        np.float32)
    return outv, results


def kernel(**inputs):
    outv, _ = run(inputs, trace=False)
    return outv


# revision 13
# speedup vs baseline: 1.0972x; 1.0056x over previous
"""KNIFE entropy regularizer loss on 8 Trainium2 NeuronCores.

reference math (per token n, center k):
    dist_sq[n,k] = max(||x_n||^2 + ||c_k||^2 - 2 x_n.c_k, 0)
    kv[n,k]      = exp(-dist_sq / (2 s_k^2))
    density[n]   = sum_k w_k kv[n,k]
    h            = -mean_n log(density + EPS)
    out          = [BETA*h, (h-TGT)^2, BETA*h + (h-TGT)^2, h]

Sharding: data-parallel over the flattened token axis N = B*S = 8192,
1024 tokens per core.  Each core receives its token shard pre-transposed
to [H=1024, T=1024] so the contraction axis (H) lands on SBUF partitions
— every DMA row is a contiguous 4KB run and the PE contracts over H
directly.  The tiny kernel params are replicated (centers pre-packed on
the host into the [128, 8*10] chunk layout the PE weights want).

Device pipeline per core:
  - 8 SWDGE cast-DMAs: xT chunk [128h, 1024t] fp32 -> bf16 SBUF
  - DVE: square (bf16)
  - PE:  psum[10,1024] += (-2c)^T_chunk @ x_chunk  and  ones^T @ x^2_chunk
         (the ones-matmul broadcasts ||x||^2 into all 10 k-rows, fusing
         the x^2 term into the same accumulator)
  - DVE: dist = max(psum + csq_k, 0)   (csq per-partition scalar)
  - ACT: kv = exp(dist * (-1/(2 s_k^2)))  -> bf16
  - PE:  density[1,1024] = w^T @ kv       (bf16 weights)
  - ACT: ln(density + EPS) with fused free-axis accumulation
  - DMA out: one fp32 partial sum per core
The epilogue runs per 512-token half so it overlaps the other half's
matmuls.  Host reduces the 8 partials and forms the 4 output scalars.
"""

from contextlib import ExitStack

import numpy as np

import concourse.bass as bass
import concourse.tile as tile
from concourse import bacc, mybir
from concourse.bass_utils import run_bass_kernel_spmd

B, S, H, K = 4, 2048, 1024, 10
N = B * S                      # 8192 tokens
NCORES = 8
TPC = N // NCORES              # 1024 tokens per core
HCHUNKS = H // 128             # 8 chunks of 128 partitions
HALF = 512                     # tokens per PSUM bank / epilogue slice
BETA = 1.0
TARGET_ENTROPY = 0.0
EPS = 1e-8

F32 = mybir.dt.float32
BF16 = mybir.dt.bfloat16


def _build_program():
    nc = bacc.Bacc("TRN2", target_bir_lowering=False, debug=False,
                   num_devices=NCORES)

    xT = nc.dram_tensor("xT", [H, TPC], F32, kind="ExternalInput").ap()
    cTp = nc.dram_tensor("cTp", [128, HCHUNKS * K], F32,
                         kind="ExternalInput").ap()
    wv = nc.dram_tensor("wv", [K, 1], F32, kind="ExternalInput").ap()
    sv = nc.dram_tensor("sv", [K, 1], F32, kind="ExternalInput").ap()
    out = nc.dram_tensor("out", [1, 1], F32, kind="ExternalOutput").ap()

    with tile.TileContext(nc) as tc, ExitStack() as ctx:
        _emit(tc, ctx, xT, cTp, wv, sv, out)
    nc.compile()
    return nc


def _emit(tc, ctx, xT, cTp, wv, sv, out):
    nc = tc.nc
    singles = ctx.enter_context(tc.tile_pool(name="singles", bufs=1))
    xbpool = ctx.enter_context(tc.tile_pool(name="xb", bufs=HCHUNKS))
    sqpool = ctx.enter_context(tc.tile_pool(name="sq", bufs=HCHUNKS))
    psum = ctx.enter_context(tc.tile_pool(name="ps", bufs=1, space="PSUM"))

    nhalf = TPC // HALF

    # ---- tiny params (SWDGE on otherwise-idle GpSimd so sync/scalar can
    # start the big x loads immediately) ----
    ct_sb = singles.tile([128, HCHUNKS, K], F32)      # [p, j, k] host-packed
    nc.gpsimd.dma_start(ct_sb[:], cTp.rearrange("p (j k) -> p j k", k=K))
    w_sb = singles.tile([K, 1], F32)
    nc.gpsimd.dma_start(w_sb[:], wv[:, :])
    s_sb = singles.tile([K, 1], F32)
    nc.gpsimd.dma_start(s_sb[:], sv[:, :])

    # ---- constants ----
    ones_bf = singles.tile([128, K], BF16)
    nc.vector.memset(ones_bf[:], 1.0)
    ones_f1 = singles.tile([128, 1], F32)
    nc.vector.memset(ones_f1[:], 1.0)
    zero_k = singles.tile([K, 1], F32)
    nc.vector.memset(zero_k[:], 0.0)
    eps_sb = singles.tile([1, 1], F32)
    nc.vector.memset(eps_sb[:], EPS)

    # ---- derived params (all tiny; off the hot path) ----
    c2_bf = singles.tile([128, HCHUNKS, K], BF16)     # -2c as bf16 weights
    nc.vector.tensor_scalar_mul(c2_bf[:], ct_sb[:], -2.0)
    w_bf = singles.tile([K, 1], BF16)
    nc.vector.tensor_copy(w_bf[:], w_sb[:])

    # -1/(2 s^2) per-partition scalar
    s2 = singles.tile([K, 1], F32)
    nc.vector.tensor_mul(s2[:], s_sb[:], s_sb[:])
    nc.vector.tensor_scalar_mul(s2[:], s2[:], 2.0)
    ninv = singles.tile([K, 1], F32)
    nc.vector.reciprocal(ninv[:], s2[:])
    nc.vector.tensor_scalar_mul(ninv[:], ninv[:], -1.0)

    # c_sq[k] = sum_h c[k,h]^2 -> [K,1] per-partition scalar
    sqc = singles.tile([128, HCHUNKS * K], F32)
    ct_flat = ct_sb.rearrange("p j k -> p (j k)")
    nc.vector.tensor_mul(sqc[:], ct_flat, ct_flat)
    ps_csq = psum.tile([1, HCHUNKS * K], F32)
    nc.tensor.matmul(ps_csq[:], lhsT=ones_f1[:], rhs=sqc[:],
                     start=True, stop=True)
    csq_row = singles.tile([1, K], F32)
    nc.vector.tensor_reduce(
        csq_row[:], ps_csq.rearrange("p (j k) -> p k j", j=HCHUNKS),
        axis=mybir.AxisListType.X, op=mybir.AluOpType.add)
    ps_csqT = psum.tile([K, 1], F32)
    nc.tensor.matmul(ps_csqT[:], lhsT=csq_row[:], rhs=ones_f1[0:1, 0:1],
                     start=True, stop=True)
    csqT = singles.tile([K, 1], F32)
    nc.scalar.copy(csqT[:], ps_csqT[:])

    # ---- stream x in fp32 over HWDGE; all dma issues emitted before any
    # engine compute (no head-of-line blocking on the issuing engines).
    # Early-arriving chunks cast on ACT (slower, idle engine); the last
    # chunks cast on DVE so the post-wire tail chain is short ----
    xtpool = ctx.enter_context(tc.tile_pool(name="xt", bufs=HCHUNKS))
    xt = []
    for j in range(HCHUNKS):
        xtj = xtpool.tile([128, TPC], F32)
        dma_eng = nc.sync if j % 2 == 0 else nc.scalar
        dma_eng.dma_start(xtj[:], xT[j * 128:(j + 1) * 128, :])
        xt.append(xtj)
    xb = []
    sq = []
    for j in range(HCHUNKS):
        xbj = xbpool.tile([128, TPC], BF16)
        if j < 4:
            nc.scalar.copy(xbj[:], xt[j][:])
        else:
            nc.vector.tensor_copy(xbj[:], xt[j][:])
        sqj = sqpool.tile([128, TPC], BF16)
        nc.vector.tensor_mul(sqj[:], xbj[:], xbj[:])
        xb.append(xbj)
        sq.append(sqj)

    # ---- main accumulation: psum[k, t] = x_sq[t] - 2 dot[k, t] ----
    ps_dist = psum.tile([K, TPC], F32)
    for h in range(nhalf):
        sl = slice(h * HALF, (h + 1) * HALF)
        for j in range(HCHUNKS):
            nc.tensor.matmul(ps_dist[:, sl], lhsT=c2_bf[:, j, :],
                             rhs=xb[j][:, sl], start=(j == 0), stop=False)
            nc.tensor.matmul(ps_dist[:, sl], lhsT=ones_bf[:],
                             rhs=sq[j][:, sl], start=False,
                             stop=(j == HCHUNKS - 1))

    # ---- per-half epilogue ----
    kv = []
    for h in range(nhalf):
        sl = slice(h * HALF, (h + 1) * HALF)
        dist = singles.tile([K, HALF], F32, tag=f"dist{h}")
        nc.vector.tensor_scalar(dist[:], ps_dist[:, sl], scalar1=csqT[:],
                                scalar2=0.0, op0=mybir.AluOpType.add,
                                op1=mybir.AluOpType.max)
        kvh = singles.tile([K, HALF], BF16, tag=f"kv{h}")
        nc.scalar.activation(kvh[:], dist[:], mybir.ActivationFunctionType.Exp,
                             bias=zero_k[:], scale=ninv[:])
        kv.append(kvh)

    ps_dens = psum.tile([1, TPC], F32)
    for h in range(nhalf):
        sl = slice(h * HALF, (h + 1) * HALF)
        nc.tensor.matmul(ps_dens[:, sl], lhsT=w_bf[:], rhs=kv[h][:],
                         start=True, stop=True)

    ld = singles.tile([1, TPC], F32)
    lsums = singles.tile([1, nhalf], F32)
    for h in range(nhalf):
        sl = slice(h * HALF, (h + 1) * HALF)
        nc.scalar.activation(ld[:, sl], ps_dens[:, sl],
                             mybir.ActivationFunctionType.Ln,
                             bias=eps_sb[:], accum_out=lsums[:, h:h + 1])
    ldsum = singles.tile([1, 1], F32)
    nc.vector.tensor_reduce(ldsum[:], lsums[:], axis=mybir.AxisListType.X,
                            op=mybir.AluOpType.add)
    nc.sync.dma_start(out[:, :], ldsum[:])


def _make_in_maps(hidden_states, kernel_centers, kernel_weights, kernel_scales):
    h_flat = np.asarray(hidden_states, dtype=np.float32).reshape(N, H)
    c = np.asarray(kernel_centers, np.float32)
    # [p, j, k] chunk layout: cTp[p, j*K+k] = c[k, j*128+p]
    cTp = np.ascontiguousarray(
        c.T.reshape(HCHUNKS, 128, K).transpose(1, 0, 2).reshape(128,
                                                                HCHUNKS * K))
    wv = np.asarray(kernel_weights, np.float32).reshape(K, 1)
    sv = np.asarray(kernel_scales, np.float32).reshape(K, 1)
    in_maps = []
    for core in range(NCORES):
        shard = h_flat[core * TPC:(core + 1) * TPC, :]    # [TPC, H]
        in_maps.append({
            "xT": np.ascontiguousarray(shard.T),          # [H, TPC]
            "cTp": cTp,
            "wv": wv,
            "sv": sv,
        })
    return in_maps


def run(inputs, trace=False, **run_kwargs):
    """Compile + run on 8 cores. Returns (output[4], BassKernelResults)."""
    nc = _build_program()
    in_maps = _make_in_maps(**inputs)
    results = run_bass_kernel_spmd(
        nc, in_maps, core_ids=list(range(NCORES)), trace=trace, **run_kwargs)
    partial = np.float32(0.0)
    for r in results.results:
        partial += np.float32(r["out"][0, 0])
    h = np.float32(-(partial / np.float32(N)))
    entropy_loss = np.float32(BETA) * h
    target_entropy_loss = np.float32((h - TARGET_ENTROPY) ** 2)
    total_loss = entropy_loss + target_entropy_loss
    outv = np.stack([entropy_loss, target_entropy_loss, total_loss, h]).astype(
        np.float32)
    return outv, results


def kernel(**inputs):
    outv, _ = run(inputs, trace=False)
    return outv
